# revision 6
# baseline (speedup 1.0000x reference)
"""DiffGLCM Trainium2 kernel v2: paired 2-slot stationary (M=128) matmuls.

Derivation (see v1 docstring for the diagonal layout): per image, pixels are
repacked host-side into diagonals: partition p holds diagonals D===p (mod 128),
slots s=0..515 along the free dim with a sentinel (x=-100, all-A-rows==0)
before each segment; the (1,1)-offset pair becomes (s, s+1). The kernel
computes the cumulative co-occurrence S[i,j] = sum_pairs A_c[i]*A_p[j] with
A_k = sigmoid(640(x - sv[k])), sv = [-10, 1/64..63/64] (row 0 is the exact
ones/data-mask row, row 64 === 0 is implicit); host applies the 2nd difference
and per-image normalization in fp64.

v2 matmul scheme: slots are split host-side into classes
  even slots 2v   -> A_even [128, 64, 259] (bins-major; col 258 = sentinel pad)
  odd  slots 4u+1 -> A_odd rows 0..63, 4u+3 -> rows 64..127: [128, 128, 129]
One matmul per u (129/image): stationary A_odd[:, :, u] (M=128: TWO odd slots
x 64 bins), moving A_even[:, :, 2u:2u+3] (N=192: THREE even slots x 64 bins).
Each matmul covers FOUR pixel pairs (4u..4u+3) vs 2 pairs/195-cols in v1:
PE streamed columns drop 2.03x (50310 -> 24768 per image) and matmul count
halves (516 -> 258 per core). Of the 6 (h,w) 64x64 output blocks, 4 are the
S / S^T partial sums and 2 are discarded. 4 PSUM accumulation chains bound
fp32 accumulation error; host sums chains in fp64.

Elementwise (unchanged math from v1, new layouts): per chunk, arg = x - sv
broadcast-sub split DVE/GPSIMD (fp32), sigmoid on ACT (~0.83ns/elem, the
bottleneck engine at ~55us/core; PE now ~40us, overlapped under it).
"""

import sys

sys.path.insert(0, "/opt/trn_rl_repo")

import numpy as np

import concourse.bass as bass
import concourse.mybir as mybir
import concourse.tile as tile
from concourse.bass_utils import run_bass_kernel_spmd

F32 = mybir.dt.float32
BF16 = mybir.dt.bfloat16
H = W = 256
NIMG = 2
NG = 64
S_TOT = 516       # 512 data slots + sentinels, uniform across partitions
NE = 259          # even-slot columns (258 even slots + 1 zero pad col)
NU = 129          # stationary groups per image (odd slot pairs)
N_ACC = 4
SENT = -100.0
EC = [43, 43, 43, 43, 43, 44]   # even v-chunks (sum 259)
OC = [22, 22, 22, 21, 21, 21]   # odd u-chunks (sum 129)
DVE_COLS_E = 32   # DVE/GPSIMD column split inside an even chunk
DVE_COLS_O = 16   # ... inside an odd chunk


def _acc_of(u):
    return 2 * (u >= 65) + (u & 1)


def _build_program(split=True, mm_dtype=BF16, loop_reps=0):
    import contextlib

    nc = bass.Bass()
    xe = nc.declare_dram_parameter("xe", [NIMG, 128, NE], F32, isOutput=False)
    xo = nc.declare_dram_parameter("xo", [NIMG, 128, 2, NU], F32, isOutput=False)
    sh_e = nc.declare_dram_parameter("sh_e", [128, NG], F32, isOutput=False)
    sh_o = nc.declare_dram_parameter("sh_o", [128, 2, NG], F32, isOutput=False)
    out = nc.declare_dram_parameter(
        "glcm", [NIMG, 128, N_ACC, 192], F32, isOutput=True
    )

    totals = [0] * N_ACC
    for u in range(NU):
        totals[_acc_of(u)] += 1

    with tile.TileContext(nc) as tc:
        with (
            tc.tile_pool(name="const", bufs=1) as const_pool,
            tc.tile_pool(name="xp", bufs=2) as x_pool,
            tc.tile_pool(name="arg", bufs=2) as arg_pool,
            tc.tile_pool(name="sig", bufs=2) as sig_pool,
            tc.tile_pool(name="oub", bufs=2) as out_pool,
            tc.tile_pool(name="ps", bufs=2, space="PSUM") as psum_pool,
        ):
            she_raw = const_pool.tile([128, NG], F32)
            nc.sync.dma_start(she_raw[:], sh_e[:])
            she = const_pool.tile([128, NG], F32)
            nc.vector.tensor_copy(she[:], she_raw[:])
            sho_raw = const_pool.tile([128, 2, NG], F32)
            nc.sync.dma_start(sho_raw[:], sh_o[:])
            sho = const_pool.tile([128, 2, NG], F32)
            nc.vector.tensor_copy(sho[:], sho_raw[:])

            rep_ctx = (
                tc.For_i(0, loop_reps, 1) if loop_reps else contextlib.nullcontext()
            )
            with rep_ctx:
              for img in range(NIMG):
                psums = [
                    psum_pool.tile([128, 192], F32, tag=f"ps{g}", name=f"ps{g}")
                    for g in range(N_ACC)
                ]
                acc_mm = [0] * N_ACC

                xer = x_pool.tile([128, NE], F32, tag="xer", name="xer")
                nc.sync.dma_start(xer[:], xe[img])
                xet = x_pool.tile([128, NE], F32, tag="xet", name="xet")
                nc.vector.tensor_copy(xet[:], xer[:])
                xor_ = x_pool.tile([128, 2, NU], F32, tag="xor", name="xor")
                nc.sync.dma_start(xor_[:], xo[img])
                xot = x_pool.tile([128, 2, NU], F32, tag="xot", name="xot")
                nc.vector.tensor_copy(xot[:], xor_[:])

                Ae = sig_pool.tile([128, NG, NE], mm_dtype, tag="Ae", name="Ae")
                Ao = sig_pool.tile([128, 2, NG, NU], mm_dtype, tag="Ao", name="Ao")

                def even_chunk(j):
                    v0 = sum(EC[:j])
                    vc = EC[j]
                    arg = arg_pool.tile([128, NG, vc], F32, tag="arge", name="arge")
                    for (lo, hi, eng) in (
                        (0, DVE_COLS_E, nc.vector),
                        (DVE_COLS_E, vc, nc.gpsimd),
                    ):
                        ncol = hi - lo
                        xb = (
                            xet[:, v0 + lo : v0 + hi]
                            .unsqueeze(1)
                            .broadcast_to([128, NG, ncol])
                        )
                        shb = she[:, :].unsqueeze(2).broadcast_to([128, NG, ncol])
                        eng.tensor_sub(arg[:, :, lo:hi], xb, shb)
                    nc.scalar.activation(
                        Ae[:, :, v0 : v0 + vc],
                        arg[:, :, 0:vc],
                        mybir.ActivationFunctionType.Sigmoid,
                        scale=640.0,
                    )

                def odd_chunk(j):
                    u0 = sum(OC[:j])
                    uc = OC[j]
                    arg = arg_pool.tile([128, 2, NG, uc], F32, tag="argo", name="argo")
                    for (lo, hi, eng) in (
                        (0, DVE_COLS_O, nc.vector),
                        (DVE_COLS_O, uc, nc.gpsimd),
                    ):
                        ncol = hi - lo
                        xb = (
                            xot[:, :, u0 + lo : u0 + hi]
                            .unsqueeze(2)
                            .broadcast_to([128, 2, NG, ncol])
                        )
                        shb = (
                            sho[:, :, :]
                            .unsqueeze(3)
                            .broadcast_to([128, 2, NG, ncol])
                        )
                        eng.tensor_sub(arg[:, :, :, lo:hi], xb, shb)
                    nc.scalar.activation(
                        Ao[:, :, :, u0 : u0 + uc],
                        arg[:, :, :, 0:uc],
                        mybir.ActivationFunctionType.Sigmoid,
                        scale=640.0,
                    )

                def mm_batch(ulo, uhi):
                    for u in range(ulo, uhi):
                        acc = _acc_of(u)
                        nc.tensor.matmul(
                            psums[acc][:, :],
                            Ao[0:128, :, :, u],
                            Ae[0:128, :, 2 * u : 2 * u + 3],
                            start=(acc_mm[acc] == 0),
                            stop=(acc_mm[acc] == totals[acc] - 1),
                        )
                        acc_mm[acc] += 1

                even_chunk(0)
                even_chunk(1)
                odd_chunk(0)
                mm_batch(0, 22)
                even_chunk(2)
                odd_chunk(1)
                mm_batch(22, 44)
                even_chunk(3)
                odd_chunk(2)
                mm_batch(44, 66)
                even_chunk(4)
                odd_chunk(3)
                mm_batch(66, 87)
                even_chunk(5)
                odd_chunk(4)
                mm_batch(87, 108)
                odd_chunk(5)
                mm_batch(108, NU)

                ob = out_pool.tile([128, N_ACC, 192], F32, name="ob")
                for g in range(N_ACC):
                    nc.vector.tensor_copy(ob[:, g, :], psums[g][:, :])
                nc.sync.dma_start(out[img], ob[:])
    if split:
        _split_waits(nc)
    return nc


def _split_waits(nc):
    n = 0
    for bb in nc.m.functions[0].blocks:
        out = []
        for ins in bb.instructions:
            si = ins.sync_info
            if si is not None and si.on_wait and len(si.on_wait) > 1:
                waits = list(si.on_wait)
                for w in waits[:-1]:
                    out.append(
                        mybir.InstDrain(
                            name=f"waitsplit-{n}",
                            engine=ins.engine,
                            sync_info=mybir.SyncInfo(on_wait=[w], on_update=[]),
                        )
                    )
                    n += 1
                ins.sync_info = mybir.SyncInfo(
                    on_wait=waits[-1:], on_update=list(si.on_update or [])
                )
            out.append(ins)
        bb.instructions[:] = out
    return n


def _shift_vec():
    sv = np.arange(0, NG, dtype=np.float64) / np.float64(NG)
    sv[0] = -10.0
    return sv


def _diag_plan():
    plan = np.full((128, S_TOT), -1, dtype=np.int64)
    for p in range(128):
        pos = 0
        for D in range(-255, 256):
            if D % 128 != p:
                continue
            cmin, cmax = max(0, -D), min(W - 1, W - 1 - D)
            pos += 1  # sentinel before each segment
            for c in range(cmin, cmax + 1):
                plan[p, pos] = (D + c) * W + c
                pos += 1
        assert pos <= S_TOT, pos
    return plan


_PLAN = _diag_plan()
_PLAN_MASK = _PLAN >= 0
_PLAN_IDX = np.where(_PLAN_MASK, _PLAN, 0)


def _to_diag(x):
    flat = x.reshape(x.shape[0], H * W)
    g = flat[:, _PLAN_IDX.reshape(-1)].reshape(x.shape[0], 128, S_TOT)
    g = np.where(_PLAN_MASK[None], g, np.float32(SENT))
    return np.ascontiguousarray(g.astype(np.float32))


def make_in_maps(x):
    sv = _shift_vec().astype(np.float32)
    sh_e = np.ascontiguousarray(np.broadcast_to(sv[None, :], (128, NG)))
    sh_o = np.ascontiguousarray(
        np.broadcast_to(sv[None, None, :], (128, 2, NG))
    )
    xd = _to_diag(x)  # [16, 128, 516]
    # even slots 0,2,..,514 plus a sentinel pad col; odd slots 4u+1 / 4u+3
    xe = np.full((16, 128, NE), np.float32(SENT), dtype=np.float32)
    xe[:, :, :258] = xd[:, :, 0::2]
    xo = np.empty((16, 128, 2, NU), dtype=np.float32)
    xo[:, :, 0, :] = xd[:, :, 1::4]
    xo[:, :, 1, :] = xd[:, :, 3::4]
    return [
        {
            "xe": np.ascontiguousarray(xe[2 * k : 2 * k + 2]),
            "xo": np.ascontiguousarray(xo[2 * k : 2 * k + 2]),
            "sh_e": sh_e,
            "sh_o": sh_o,
        }
        for k in range(8)
    ]


def _finish_host(raw):
    # raw: [16, 128, N_ACC, 192]
    r = raw.astype(np.float64).sum(axis=2)  # [16, 128, 192]
    B = {}
    for h in range(2):
        for w in range(3):
            B[(h, w)] = r[:, 64 * h : 64 * h + 64, w::3]
    # pairs: (h0,w0) S^T, (h0,w1) S, (h1,w1) S^T, (h1,w2) S
    S = B[(0, 1)] + B[(1, 2)] + np.transpose(B[(0, 0)] + B[(1, 1)], (0, 2, 1))
    Sp = np.zeros((16, 65, 65))
    Sp[:, :64, :64] = S
    g = Sp[:, :64, :64] - Sp[:, 1:, :64] - Sp[:, :64, 1:] + Sp[:, 1:, 1:]
    g = g / g.sum(axis=(1, 2), keepdims=True)
    return g.astype(np.float32)


def _postprocess(results):
    raw = np.concatenate([r["glcm"] for r in results], axis=0)
    return _finish_host(raw).reshape(16, 1, NG, NG, 1)


_NC = None


def kernel(x, offset_r=1, offset_c=1, **_):
    global _NC
    assert int(offset_r) == 1 and int(offset_c) == 1
    x = np.ascontiguousarray(np.asarray(x, dtype=np.float32).reshape(16, H, W))
    if _NC is None:
        _NC = _build_program()
    res = run_bass_kernel_spmd(_NC, make_in_maps(x), core_ids=list(range(8)))
    return _postprocess(res.results)


if __name__ == "__main__":
    _build_program()
    print("build OK")
